# revision 61
# baseline (speedup 1.0000x reference)
"""Trainium2 Bass kernel for block-local (sliding-window) GQA attention with RoPE.

Module: x:[1,4096,2048] -> Q/K/V proj -> RoPE -> block-local attention
(window W=1024, block 1024, GQA 16 q-heads / 4 kv-heads, D=128) -> out proj.

Sharding: sequence-parallel over 8 cores, 512 queries per core. Each core
recomputes K/V for its 1536-row key span (queries + 1024 history, zero-padded
at the left edge), runs attention for all 16 heads on its query chunk, and
applies the full output projection locally; outputs concatenate over T.

Layout: feature-major ("transposed") activations; all matmuls in float32r
with moving-dim >= 256 (full PE rate; below 256 f32r runs at 1/4 rate).
Attention processes all 4 q-heads of a kv-group per matmul (N=512 moving),
which also hides LDWEIGHTS on hardware. Softmax uses exp without
max-subtraction, masks via additive [128,512] triangle constants, and
denominators via an all-ones stationary matmul; the padfix constant corrects
core 0's zero-padded history keys.

Schedule: wv streams on the Pool SWDGE queue while xh groups alternate the
SP/ACT HWDGE queues (the 12MB of V-phase input cannot keep up with pass-1
compute through one queue); group 0 is split per-ct (first xh ct halved) so
the first V matmul starts ~2.4us in. Dummy f32 matmuls on zeroed SBUF warm
the PE p-state ramp and a tiny activation preloads the ACT function table
during the initial DMA wait. V pass 1 runs ct-outer; its high-bank drain
copies split across DVE/ACT; pass 2 accumulates tt-outer straight into the
drained high-bank tiles. Then K, then Q (separate PSUM pools per phase),
then software-pipelined attention (scores/exp of group i+1 issue before the
su/at chains of group i, su/at interleaved per key-chunk; the sa PSUM pool
enters only after the first scores issue). Attention mask constants load in
the early head stream; the padfix constant reuses wk0's right-side SBUF
after K retires. wo streams during attention into a 6-deep pool; the final
out-proj chunk computes/copies/DMAs in column halves (own PSUM tiles, last
copy on DVE) to halve the post-PE drain tail.

su keeps the baseline 9-chunk accumulation + padfix subtract: a DVE
pb0+pb8 pre-combine saves 3.4us of PE but f32r DVE adds are reduced
precision on HW and push the t=0 rows (few-key softmax, ~25x the mean
output magnitude) past the 2e-2 max/mean error gate.
"""
import os
import sys

for _p in ("/root/.axon_site", "/root/.axon_site/_ro/trn_rl_repo", "/opt/trn_rl_repo"):
    if os.path.isdir(_p) and _p not in sys.path:
        sys.path.append(_p)

import numpy as np

import concourse.bass as bass
import concourse.tile as tile
import concourse.mybir as mybir
from concourse.vector_clock import ScopedClock
from concourse.bass_utils import run_bass_kernel_spmd

dt = mybir.dt

B, T, C = 1, 4096, 2048
H, HK, D = 16, 4, 128
W = 1024
THETA = 10000.0
NCORES = 8
TQ = T // NCORES            # 512 queries per core
TK = TQ + W                 # 1536-key span per core
NQC = TQ // 128             # 4 query chunks of 128
NJ = NQC + W // 128 - 3     # 9 key chunks per query chunk
NCT = C // 128              # 16 contraction tiles
SCALE = 1.0 / float(np.sqrt(D))
NEG = -1.0e30


def _patch_tile_drain():
    """CoreV3 codegen caps sync-waits per instruction; the stock TileContext
    tail drain carries one wait per live semaphore.  Spill the waits across
    preceding sync-engine no-ops, one wait each."""
    if getattr(tile.TileContext, "_drain_patched", False):
        return

    def _drain_and_barrier(self, tick_clock, wait_clock):
        nc = self.nc
        probe = nc.sync.nop()
        wait_clock.add_sem_waits(
            probe.ins, ScopedClock({None: tick_clock.global_clock})
        )
        si = probe.ins.sync_info
        waits = list(si.on_wait) if si is not None and si.on_wait else []
        if len(waits) > 1:
            si.on_wait = waits[:1]
            for w in waits[1:]:
                extra = nc.sync.nop()
                extra.ins.sync_info = mybir.SyncInfo(on_wait=[w], on_update=[])
        nc.sync.drain()
        nc.all_engine_barrier()
        assert self.sems is not None
        popped = nc._tile_sem_poison_stack.pop()
        assert popped is self._sem_poison
        nc.clear_and_free_semaphores(list(self.sems.allocated().values()))
        nc.all_engine_barrier()

    tile.TileContext._drain_and_barrier = _drain_and_barrier
    tile.TileContext._drain_patched = True


_MAX_WAITS = 1


def _spill_excess_waits(nc):
    """Walrus codegen caps sync-waits per instruction.  For any instruction
    carrying more, move the excess onto same-engine no-ops inserted just
    before it (engines execute in program order, so the waits still resolve
    before the instruction runs)."""
    n = [0]
    for f in nc.m.functions:
        for bb in f.blocks:
            out = []
            for inst in bb.instructions:
                si = inst.sync_info
                waits = list(si.on_wait) if si is not None and si.on_wait else []
                if len(waits) > _MAX_WAITS:
                    for lo in range(0, len(waits) - _MAX_WAITS, _MAX_WAITS):
                        nop = mybir.InstNoOp(
                            name=f"waitspill-{n[0]}", ins=[], outs=[]
                        )
                        n[0] += 1
                        nop.engine = inst.engine
                        nop.sync_info = mybir.SyncInfo(
                            on_wait=waits[lo:lo + _MAX_WAITS], on_update=[]
                        )
                        out.append(nop)
                    si.on_wait = waits[len(waits) - _MAX_WAITS:]
                out.append(inst)
            bb.instructions[:] = out


def _rope_tables():
    d2 = np.arange(0, D, 2, dtype=np.float64) / D
    ts = THETA ** d2
    ang = np.arange(T, dtype=np.float64)[:, None] / ts[None, :]
    ang = np.concatenate([ang, ang], axis=1)            # [T, D]
    cosT = np.cos(ang).T                                # [D, T]
    sinS = np.sin(ang).T
    sinS[: D // 2] *= -1.0    # rot(u)[d<64] = -u[d+64]; out = u*cos + shift(u)*sinS
    return cosT.astype(np.float32), sinS.astype(np.float32)


def _build_program():
    nc = bass.Bass(num_swdge_queues=4)
    f32, f32r = dt.float32, dt.float32r
    bf16 = dt.float16

    xh_e = nc.declare_dram_parameter("xh", [C, W], f32r, isOutput=False)
    xq_e = nc.declare_dram_parameter("xq", [C, TQ], f32r, isOutput=False)
    wq_e = nc.declare_dram_parameter("wq", [C, H * D], f32r, isOutput=False)
    wk_e = nc.declare_dram_parameter("wk", [C, HK * D], f32r, isOutput=False)
    wv_e = nc.declare_dram_parameter("wv", [C, HK * D], f32r, isOutput=False)
    wo_e = nc.declare_dram_parameter("wo", [H * D, C], f32r, isOutput=False)
    cos_e = nc.declare_dram_parameter("cosk", [D, TK], f32, isOutput=False)
    sin_e = nc.declare_dram_parameter("sink", [D, TK], f32, isOutput=False)
    tri4w_e = nc.declare_dram_parameter("tri4w", [128, 512], f32, isOutput=False)
    tri4c_e = nc.declare_dram_parameter("tri4c", [128, 512], f32, isOutput=False)
    pad_e = nc.declare_dram_parameter("padfix", [128, NQC * 512], f32r, isOutput=False)
    ones_e = nc.declare_dram_parameter("onesc", [128, 128], f32r, isOutput=False)
    y_e = nc.declare_dram_parameter("y", [TQ, C], f32, isOutput=True)

    Exp = mybir.ActivationFunctionType.Exp
    NG = 4                    # ct tiles per DMA group
    NGRP = NCT // NG          # 4 groups

    with tile.TileContext(nc) as tc:
        with (
            tc.tile_pool(name="consts", bufs=1) as cst,
            tc.tile_pool(name="vout", bufs=1) as vout,
            tc.tile_pool(name="krp", bufs=1) as krp,
        ):
            tri4w = cst.tile([128, 512], f32, tag="tri4w")
            tri4c = cst.tile([128, 512], f32, tag="tri4c")
            ones = cst.tile([128, 128], f32r, tag="ones")
            scr1 = cst.tile([1, 8], f32, tag="scr1")

            # warm the ACT function table during the initial DMA wait — the
            # first real activation otherwise pays the ~1.3us table load on
            # the V-phase critical path
            nc.vector.memset(scr1[0:1, 0:8], 0.0)
            nc.scalar.activation(scr1[0:1, 0:4], scr1[0:1, 4:8], Exp)
            scrM = cst.tile([128, 128], f32, tag="scrM")
            nc.vector.memset(scrM[:], 0.0)

            v_t = [vout.tile([128, HK * D], f32r, name=f"v{tt}", tag=f"v{tt}")
                   for tt in range(TK // 128)]
            kr = [krp.tile([D, TK], f32r, name=f"kr{g}", tag=f"kr{g}")
                  for g in range(HK)]
            an = {}

            # trig tables live until the end of Q-RoPE
            trig_cm = tc.tile_pool(name="trig", bufs=1)
            trig = trig_cm.__enter__()
            cosk = trig.tile([D, TK], f32, tag="cosk")
            sink = trig.tile([D, TK], f32, tag="sink")

            # ------- input streams: wv, xh groups, xq groups, wk, wq, wo ---
            # wk slab g0 loads early via the ACT hardware-DGE queue (its own
            # completion counter, so K's first matmul doesn't conflate with
            # the space-blocked g1-3 slabs on the SP queue)
            wkr_cm = tc.tile_pool(name="wkr", bufs=1, side="right")
            wkr = wkr_cm.__enter__()
            wk0 = wkr.tile([128, C], f32r, name="wk0", tag="wk0")

            xqp_cm = tc.tile_pool(name="xqp", bufs=1)
            xqp = xqp_cm.__enter__()
            xhp_cm = tc.tile_pool(name="xhp", bufs=1)
            xhp = xhp_cm.__enter__()
            wvp_cm = tc.tile_pool(name="wvp", bufs=1)
            wvp = wvp_cm.__enter__()

            xh_g = [xhp.tile([128, NG * W], f32r, name=f"xh{g2}", tag=f"xh{g2}")
                    for g2 in range(NGRP)]
            xq_g = [xqp.tile([128, NG * TQ], f32r, name=f"xq{g2}", tag=f"xq{g2}")
                    for g2 in range(NGRP)]
            wv_g = [wvp.tile([128, NG * HK * D], f32r, name=f"wv{g2}",
                             tag=f"wv{g2}") for g2 in range(NGRP)]

            def grp_dma(dst, src_e, g2, eng=None, lo=0, n=NG):
                # transfer ct tiles [lo, lo+n) of group g2
                rows = slice(128 * (NG * g2 + lo), 128 * (NG * g2 + lo + n))
                src = src_e[rows, :].rearrange("(a p) m -> p a m", p=128)
                m = dst.shape[-1] // NG
                (eng or nc.gpsimd).dma_start(
                    dst[:, m * lo:m * (lo + n)].rearrange(
                        "p (a m) -> p a m", a=n
                    ), src
                )

            # head: wv on the Pool SWDGE queue, xh in parallel on the SP
            # HWDGE queue; group 0 split per-ct so the first V matmul can
            # start as soon as ~1.5MB lands instead of 3MB; the very first
            # xh ct is halved again (tt 0-3 matmuls only need cols 0:512)
            for ct0 in range(NG):
                grp_dma(wv_g[0], wv_e, 0, lo=ct0, n=1)
                if ct0 == 0:
                    src = xh_e[0:128, :]
                    nc.sync.dma_start(xh_g[0][:, 0:512], src[:, 0:512])
                    nc.sync.dma_start(xh_g[0][:, 512:W], src[:, 512:W])
                else:
                    grp_dma(xh_g[0], xh_e, 0, eng=nc.sync, lo=ct0, n=1)
            for g2 in range(1, NGRP):
                grp_dma(wv_g[g2], wv_e, g2)
                # alternate xh groups across the SP and ACT HWDGE queues:
                # 8MB of xh through one queue (~25us) can't keep up with the
                # 27us of pass-1 compute once the fine-grained head is burned
                grp_dma(xh_g[g2], xh_e, g2,
                        eng=nc.sync if g2 % 2 == 0 else nc.scalar)
            for g2 in range(NGRP):
                grp_dma(xq_g[g2], xq_e, g2)
            # attention constants: tiny, land early on the Pool queue so the
            # first mask-adds never wait (they used to land ~149us in)
            nc.gpsimd.dma_start(tri4w[:], tri4w_e[:])
            nc.gpsimd.dma_start(tri4c[:], tri4c_e[:])
            nc.gpsimd.dma_start(ones[:], ones_e[:])
            # gate the early wk0 HWDGE transfer behind the x stream (it has
            # no natural dependency and would otherwise steal head bandwidth)
            nc.vector.tensor_copy(wk0[0:1, 0:1], xq_g[NGRP - 1][0:1, 0:1])

            def xh_slice(ct, lo, size):
                base = W * (ct % NG)
                return xh_g[ct // NG][:, base + lo:base + lo + size]

            def xq_slice(ct, lo, size):
                base = TQ * (ct % NG)
                return xq_g[ct // NG][:, base + lo:base + lo + size]

            def xk_slice(ct, lo, size):
                if lo + size <= W:
                    return xh_slice(ct, lo, size)
                assert lo >= W
                return xq_slice(ct, lo - W, size)

            def wv_slice(ct):
                base = HK * D * (ct % NG)
                return wv_g[ct // NG][:, base:base + HK * D]

            # ---- V projection: ct-outer, two PSUM passes ------------------
            vlo_cm = tc.tile_pool(name="vlo", bufs=1, space="PSUM")
            vlo = vlo_cm.__enter__()
            vhi_cm = tc.tile_pool(name="vhi", bufs=1, space="PSUM")
            vhi = vhi_cm.__enter__()
            ps8 = ([vlo.tile([128, HK * D], dt.float32, name=f"vp{tt}",
                             tag=f"vp{tt}") for tt in range(4)]
                   + [vhi.tile([128, HK * D], dt.float32, name=f"vp{tt}",
                               tag=f"vp{tt}") for tt in range(4, 8)])
            # PE p-state warmup: dummy f32 matmuls on zeroed SBUF during the
            # initial DMA wait, so the ramp-to-full-clock (3us of continuous
            # execution) burns on throwaway work instead of real V matmuls
            for _w in range(5):
                nc.tensor.matmul(
                    ps8[0][0:64, 0:64], scrM[:, 0:64], scrM[:, 64:128],
                    start=True, stop=True, skip_group_check=True,
                )
            for ct in range(NCT):
                for tt in range(8):
                    nc.tensor.matmul(
                        ps8[tt][:], xh_slice(ct, 128 * tt, 128),
                        wv_slice(ct),
                        start=(ct == 0), stop=(ct == NCT - 1),
                    )
            # drain the high banks (pass 2's space) FIRST, split across DVE
            # and ACT so the drain takes 2 copy-times, not 4 serial ones
            for tt in (4, 6, 5, 7, 0, 2, 1, 3):
                if tt % 2 == 0:
                    nc.vector.tensor_copy(v_t[tt][:], ps8[tt][:])
                else:
                    nc.scalar.copy(v_t[tt][:], ps8[tt][:])
                if tt == 7:
                    nc.scalar.dma_start(
                        wk0[:].rearrange("p (a m) -> p a m", a=NCT),
                        wk_e[:, 0:128].rearrange("(a p) m -> p a m", p=128),
                    )
            # V pass 2 accumulates straight into the (drained) high-bank
            # tiles — tile-level dependency tracking lets each chain start as
            # soon as ITS bank's drain copy is done, instead of gating a new
            # pool on all four drains. tt-outer so the first chain runs on
            # the DVE-drained bank while the serial ACT copies catch up.
            for tt in range(4):
                for ct in range(NCT):
                    nc.tensor.matmul(
                        ps8[4 + tt][:], xq_slice(ct, 128 * tt, 128),
                        wv_slice(ct),
                        start=(ct == 0), stop=(ct == NCT - 1),
                    )
                if tt % 2 == 0:
                    nc.vector.tensor_copy(v_t[8 + tt][:], ps8[4 + tt][:])
                else:
                    nc.scalar.copy(v_t[8 + tt][:], ps8[4 + tt][:])
            vhi_cm.__exit__(None, None, None)
            vlo_cm.__exit__(None, None, None)

            wvp_cm.__exit__(None, None, None)

            # ---- K projection + RoPE: krT[d, t] ---------------------------
            wkm_cm = tc.tile_pool(name="wkm", bufs=1)
            wkm = wkm_cm.__enter__()
            wk_s = [wk0]
            for g in range(1, HK):
                wslab = wkm.tile([128, C], f32r, name=f"wk{g}", tag=f"wk{g}")
                src = wk_e[:, 128 * g:128 * (g + 1)].rearrange(
                    "(a p) m -> p a m", p=128
                )
                nc.sync.dma_start(
                    wslab[:].rearrange("p (a m) -> p a m", a=NCT), src
                )
                wk_s.append(wslab)
            nc.gpsimd.dma_start(cosk[:], cos_e[:])
            nc.gpsimd.dma_start(sink[:], sin_e[:])

            pp_cm = tc.tile_pool(name="pp", bufs=4, space="PSUM")
            pp = pp_cm.__enter__()
            with tc.tile_pool(name="shf", bufs=2) as shf:
                # history chains (xh) first, query-side chains (xq) last so
                # the xh region frees early for the wq prefetch behind it
                korder = [(g, tcb) for g in range(HK) for tcb in range(2)]
                korder += [(g, 2) for g in range(HK)]
                for g, tcb in korder:
                        ps = pp.tile([128, 512], dt.float32, tag="pp")
                        for ct in range(NCT):
                            nc.tensor.matmul(
                                ps[:], wk_s[g][:, 128 * ct:128 * (ct + 1)],
                                xk_slice(ct, 512 * tcb, 512),
                                start=(ct == 0), stop=(ct == NCT - 1),
                            )
                        sl = slice(512 * tcb, 512 * (tcb + 1))
                        qs = shf.tile([128, 512], dt.float32, tag="qs")
                        nc.scalar.copy(qs[0:64, :], ps[64:128, :])
                        nc.scalar.copy(qs[64:128, :], ps[0:64, :])
                        nc.vector.tensor_mul(kr[g][:, sl], ps[:], cosk[:, sl])
                        nc.vector.tensor_mul(qs[:], qs[:], sink[:, sl])
                        nc.vector.tensor_add(kr[g][:, sl], kr[g][:, sl], qs[:])

            wkm_cm.__exit__(None, None, None)
            wkr_cm.__exit__(None, None, None)
            xhp_cm.__exit__(None, None, None)

            # negpad loads into wk0's freed right-side space after K retires
            # (needed only from attention, ~45us later)
            padfp_cm = tc.tile_pool(name="padfp", bufs=1, side="right")
            padfp = padfp_cm.__enter__()
            padf = padfp.tile([128, NQC * 512], f32r, tag="padf")
            nc.gpsimd.dma_start(padf[:], pad_e[:])

            # ---- Q projection + RoPE (query columns only) -----------------
            qpr_cm = tc.tile_pool(name="qpr", bufs=1, side="right")
            qpr = qpr_cm.__enter__()
            qp = [qpr.tile([D, NQC * 512], f32r, name=f"qp{p}", tag=f"qp{p}")
                  for p in range(HK)]
            # qp layout per kv-group: 4 qc-blocks of 512 = 4 q-heads x 128
            csl = slice(W, W + TQ)
            cosq = cosk[:, csl].rearrange("p (a m) -> p a m", a=NQC)
            sinq = sink[:, csl]
            with (
                tc.tile_pool(name="qpp", bufs=4, space="PSUM") as qpp,
                tc.tile_pool(name="wqm", bufs=4) as wqm,
                tc.tile_pool(name="shq", bufs=2) as shq,
            ):
                for m in range(H):
                    pr, half = m // 4, m % 4
                    wslab = wqm.tile([128, C], f32r, tag="wqm")
                    src = wq_e[:, 128 * m:128 * (m + 1)].rearrange(
                        "(a p) m -> p a m", p=128
                    )
                    nc.sync.dma_start(
                        wslab[:].rearrange("p (a m) -> p a m", a=NCT), src
                    )
                    ps = qpp.tile([128, TQ], dt.float32, tag="qpp")
                    for ct in range(NCT):
                        nc.tensor.matmul(
                            ps[:], wslab[:, 128 * ct:128 * (ct + 1)],
                            xq_slice(ct, 0, TQ),
                            start=(ct == 0), stop=(ct == NCT - 1),
                        )
                    qs = shq.tile([128, TQ], dt.float32, tag="qs")
                    nc.scalar.copy(qs[0:64, :], ps[64:128, :])
                    nc.scalar.copy(qs[64:128, :], ps[0:64, :])
                    nc.vector.tensor_mul(qs[:], qs[:], sinq)
                    qpv = qp[pr][:].rearrange("p (a h m) -> p a h m",
                                              a=NQC, h=4)[:, :, half, :]
                    psv = ps[:].rearrange("p (a m) -> p a m", a=NQC)
                    qsv = qs[:].rearrange("p (a m) -> p a m", a=NQC)
                    nc.vector.tensor_mul(qpv, psv, cosq)
                    nc.vector.tensor_add(qpv, qpv, qsv)

            pp_cm.__exit__(None, None, None)
            xqp_cm.__exit__(None, None, None)
            trig_cm.__exit__(None, None, None)

            # ---- wo prefetch (8 half-chunks, stream during attention) -----
            wop_cm = tc.tile_pool(name="wop", bufs=6)
            wop = wop_cm.__enter__()
            wo_hc = []
            for cc in range(4):
                for uh in range(4):
                    wt = wop.tile([128, 4 * 512], f32r, name=f"wo{cc}_{uh}",
                                  tag="wo")
                    src = wo_e[512 * uh:512 * (uh + 1),
                               512 * cc:512 * (cc + 1)].rearrange(
                        "(a p) m -> p a m", p=128
                    )
                    nc.gpsimd.dma_start(
                        wt[:].rearrange("p (a m) -> p a m", a=4), src
                    )
                    wo_hc.append(wt)

            # ---- attention ------------------------------------------------
            attnp_cm = tc.tile_pool(name="attn", bufs=1)
            attnp = attnp_cm.__enter__()
            psa_box = [None]
            psa_cm_box = [None]
            with (
                tc.tile_pool(name="sc4", bufs=2, space="PSUM") as psc4,
                tc.tile_pool(name="pb", bufs=4) as pbp,
                tc.tile_pool(name="rc", bufs=2) as rcp,
            ):
                def issue_scores(qc, g):
                    qmv = qp[g][:, 512 * qc:512 * (qc + 1)]
                    pbs = []
                    for jq in range(5):
                        njs = 2 if jq < 4 else 1
                        scq = psc4.tile([128, 1024], dt.float32, tag="sc4")
                        for s in range(njs):
                            j = 2 * jq + s
                            lk = 128 * (qc + j)
                            nc.tensor.matmul(
                                scq[:, 512 * s:512 * (s + 1)],
                                kr[g][:, lk:lk + 128], qmv,
                                start=True, stop=True, skip_group_check=True,
                            )
                        if jq == 0:
                            nc.vector.tensor_add(
                                scq[:, 0:512], scq[:, 0:512], tri4w[:]
                            )
                        if jq == 4:
                            nc.vector.tensor_add(
                                scq[:, 0:512], scq[:, 0:512], tri4c[:]
                            )
                        pbt = pbp.tile([128, 1024], f32r, tag="pb")
                        nc.scalar.activation(
                            pbt[:, 0:512 * njs], scq[:, 0:512 * njs], Exp,
                            scale=SCALE
                        )
                        pbs.append(pbt)
                    return pbs

                def issue_suat(qc, g, pbs):
                    def pb_j(j):
                        return pbs[j // 2][:, 512 * (j % 2):512 * (j % 2 + 1)]

                    suat = psa_box[0].tile([128, 1024], dt.float32, tag="sa")
                    su_ps = suat[:, 0:512]
                    at_ps = suat[:, 512:1024]
                    for j in range(NJ):
                        nc.tensor.matmul(
                            su_ps, ones[:], pb_j(j),
                            start=(j == 0), stop=(j == NJ - 1),
                            skip_group_check=True,
                        )
                        nc.tensor.matmul(
                            at_ps, v_t[qc + j][:, 128 * g:128 * (g + 1)],
                            pb_j(j),
                            start=(j == 0), stop=(j == NJ - 1),
                            skip_group_check=True,
                        )
                    rec = rcp.tile([128, 512], dt.float32, tag="rc")
                    nc.vector.tensor_sub(
                        rec[:], su_ps,
                        padf[:, 512 * qc:512 * (qc + 1)]
                    )
                    nc.vector.reciprocal(rec[:], rec[:])
                    a = attnp.tile([128, 512], f32r, name=f"an{g}_{qc}",
                                   tag=f"an{g}_{qc}")
                    an[(g, qc)] = a
                    nc.vector.tensor_mul(a[:], at_ps, rec[:])

                # software pipeline: scores/exp of group i+1 issue before the
                # su/at chains of group i, so the PE never waits on the exp.
                # psa enters only after the first scores issue so the scores
                # matmuls don't inherit psa's wait on qpp's late drain
                groups = [(qc, g) for qc in range(NQC) for g in range(HK)]
                pending = None
                for qc, g in groups:
                    pbs = issue_scores(qc, g)
                    if psa_box[0] is None:
                        psa_cm_box[0] = tc.tile_pool(name="sa", bufs=2,
                                                     space="PSUM")
                        psa_box[0] = psa_cm_box[0].__enter__()
                    if pending is not None:
                        issue_suat(*pending)
                    pending = (qc, g, pbs)
                issue_suat(*pending)
                psa_cm_box[0].__exit__(None, None, None)

            qpr_cm.__exit__(None, None, None)

            # ---- output projection ---------------------------------------
            with (
                tc.tile_pool(name="yp", bufs=2, space="PSUM") as pyp,
                tc.tile_pool(name="ych", bufs=3) as ychp,
            ):
                for cc in range(4):
                    for qc in range(NQC):
                        last = (cc == 3 and qc == NQC - 1)
                        ych = ychp.tile([128, 512], dt.float32, tag="ych")
                        # final chunk: compute/copy/DMA in column halves (own
                        # PSUM tiles, else the shared-tile hazard serializes)
                        for half in ((0, 1) if last else (0,)):
                            wd = 256 if last else 512
                            sl = slice(wd * half, wd * (half + 1))
                            ps = pyp.tile([128, wd], dt.float32, tag="yp")
                            for h in range(H):
                                nc.tensor.matmul(
                                    ps[:],
                                    an[(h // 4, qc)][:, 128 * (h % 4):128 * (h % 4 + 1)],
                                    wo_hc[4 * cc + h // 4][:, 512 * (h % 4) + wd * half:
                                                           512 * (h % 4) + wd * (half + 1)],
                                    start=(h == 0), stop=(h == H - 1),
                                    skip_group_check=last,
                                )
                            if last and half == 1:
                                nc.vector.tensor_copy(ych[:, sl], ps[:])
                            else:
                                nc.scalar.copy(ych[:, sl], ps[:])
                            nc.sync.dma_start(
                                y_e[128 * qc:128 * (qc + 1),
                                    512 * cc + wd * half:512 * cc + wd * (half + 1)],
                                ych[:, sl]
                            )
            attnp_cm.__exit__(None, None, None)
            wop_cm.__exit__(None, None, None)
            padfp_cm.__exit__(None, None, None)
    _spill_excess_waits(nc)
    return nc


def _host_inputs(x, q_kernel, k_kernel, v_kernel, out_kernel):
    x2 = np.ascontiguousarray(np.asarray(x, np.float32)[0])      # [T, C]
    xT = np.zeros((C, W + T), np.float32)
    xT[:, W:] = x2.T
    cosT, sinS = _rope_tables()
    cos_pad = np.concatenate([np.repeat(cosT[:, :1], W, axis=1), cosT], axis=1)
    sin_pad = np.concatenate([np.repeat(sinS[:, :1], W, axis=1), sinS], axis=1)

    i1 = np.arange(128)
    triw = np.where(i1[None, :] <= i1[:, None], 0.0, NEG).astype(np.float32)  # valid qi <= kj
    tric = np.where(i1[None, :] >= i1[:, None], 0.0, NEG).astype(np.float32)  # valid qi >= kj
    tri4w = np.tile(triw, (1, 4))      # four q-heads of a kv-group share it
    tri4c = np.tile(tric, (1, 4))

    wq = np.ascontiguousarray(np.asarray(q_kernel, np.float32))
    wk = np.ascontiguousarray(np.asarray(k_kernel, np.float32))
    wv = np.ascontiguousarray(np.asarray(v_kernel, np.float32))
    wo = np.ascontiguousarray(np.asarray(out_kernel, np.float32))

    in_maps = []
    for core in range(NCORES):
        q0 = TQ * core
        xk = xT[:, q0:q0 + TK]
        npad = max(0, (W - q0) // 128)
        padf = np.zeros((128, NQC * 512), np.float32)
        qi = np.arange(128, dtype=np.float32)
        for qc in range(NQC):
            pv = np.zeros(128, np.float32)
            if qc < npad:
                pv += 128.0 - qi           # j=0 window chunk: valid count #{kj >= qi}
            for j in range(1, NJ - 1):
                if qc + j < npad:
                    pv += 128.0
            padf[:, 512 * qc:512 * (qc + 1)] = np.tile(pv, 4)[None, :]
        in_maps.append({
            "xh": np.ascontiguousarray(xk[:, :W]),
            "xq": np.ascontiguousarray(xk[:, W:]),
            "wq": wq, "wk": wk, "wv": wv, "wo": wo,
            "cosk": np.ascontiguousarray(cos_pad[:, q0:q0 + TK]),
            "sink": np.ascontiguousarray(sin_pad[:, q0:q0 + TK]),
            "tri4w": tri4w, "tri4c": tri4c,
            "padfix": padf,
            "onesc": np.ones((128, 128), np.float32),
        })
    return in_maps


_CACHED = {}


def kernel(x, q_kernel, k_kernel, v_kernel, out_kernel, _profile=False):
    _patch_tile_drain()
    if "nc" not in _CACHED:
        _CACHED["nc"] = _build_program()
    nc = _CACHED["nc"]
    in_maps = _host_inputs(x, q_kernel, k_kernel, v_kernel, out_kernel)
    res = run_bass_kernel_spmd(nc, in_maps, list(range(NCORES)), trace=_profile)
    y = np.concatenate([res.results[i]["y"] for i in range(NCORES)], axis=0)
    out = y[None, :, :].astype(np.float32)
    if _profile:
        return out, res
    return out

